# revision 33
# baseline (speedup 1.0000x reference)
"""AdaptiveGravityAttention on 8 TRN2 NeuronCores (Bass/Tile, SPMD).

Sharding: heads across cores (2 heads/core, all batches), token-parallel
final projection via per-batch AllToAll. All matmuls fp16/bf16 inputs with
fp32 PSUM accumulation.

Math notes:
- softmax rows are rebased by exp(+6.4*g[i]) so the gravity bias becomes an
  additive band term 0.1*g[i]*max(64-dist,0) that is zero outside |i-j|<64;
  causal masking rides the same term via a -1e9 entry (g>0 always).
- logits are bounded (~|6|+6.4*g<=40) so exp needs no running max; exp
  outputs stored bf16 (range), q/k/v/x/W stored fp16 (precision).

Structure (v2):
- phase 1: projections + rope + V-transpose for ALL batches (keeps PE dense)
- phase 2: per batch: attention (head-paired S matmuls on disjoint PE row
  groups), batched reciprocal, normalize, AllToAll
- phase 3: output projection for this core's token slice of every batch
- SBUF-only elementwise (rope sin-term, near-band product, normalize mul)
  runs on the otherwise-idle GpSimd engine; DVE keeps all PSUM traffic.
"""
import os
import sys

sys.path.insert(0, "/opt/trn_rl_repo")

import numpy as np

import concourse.bass as bass  # noqa: F401
import concourse.mybir as mybir
import concourse.tile as tile
from concourse import bacc
from concourse.masks import make_identity
from concourse.bass_utils import run_bass_kernel_spmd

B, T, C = 4, 2048, 1024
H, D = 16, 64
NC = 8
HP = H // NC          # heads per core = 2
ROWS = HP * D         # 128 q/k/v rows per core
TB = T                # tokens per batch
QCH = 512             # q-chunk width
NQC = TB // QCH       # 4
KT = 128              # k-tile
NKT = TB // KT        # 16
PCH = 512             # projection token chunk
NPC = TB // PCH       # 4
TOK = T // NC         # tokens per core per batch for out-proj = 256

F16 = mybir.dt.float16
BF16 = mybir.dt.bfloat16
F32 = mybir.dt.float32

_CACHED = {}
LAST_RESULT = None
DEBUG = False


def build_nc():
    nc = bacc.Bacc("TRN2", target_bir_lowering=False, num_devices=NC)

    # ---- dram parameters (per-core shards prepared on host) ----
    xT = nc.declare_dram_parameter("xT", [C, B * T], F16, isOutput=False)
    rope2 = nc.declare_dram_parameter("rope2", [128, T], F32, isOutput=False)
    rope2s = nc.declare_dram_parameter("rope2s", [128, T], F32, isOutput=False)
    Wq = nc.declare_dram_parameter("Wq", [C, ROWS], F16, isOutput=False)
    Wk = nc.declare_dram_parameter("Wk", [C, ROWS], F16, isOutput=False)
    Wv = nc.declare_dram_parameter("Wv", [C, ROWS], F16, isOutput=False)
    gw = nc.declare_dram_parameter("gw", [C, HP], F16, isOutput=False)
    gb = nc.declare_dram_parameter("gb", [HP, 1], F32, isOutput=False)
    Rdiag = nc.declare_dram_parameter("Rdiag", [128, 128], F32, isOutput=False)
    Roff = nc.declare_dram_parameter("Roff", [128, 128], F32, isOutput=False)
    Wproj = nc.declare_dram_parameter("Wproj", [C, C], F16, isOutput=False)
    out = nc.declare_dram_parameter("out", [B, 8, 128, TOK], F32, isOutput=True)
    if DEBUG:
        dbg_q = nc.declare_dram_parameter("dbg_q", [128, TB], F16, isOutput=True)
        dbg_k = nc.declare_dram_parameter("dbg_k", [128, TB], F16, isOutput=True)
        dbg_g = nc.declare_dram_parameter("dbg_g", [HP, TB], F32, isOutput=True)
        dbg_yt = nc.declare_dram_parameter("dbg_yt", [NC, ROWS + HP, TOK], BF16, isOutput=True)
        dbg_s16 = nc.declare_dram_parameter("dbg_s16", [H, TOK], F32, isOutput=True)
        dbg_s16b = nc.declare_dram_parameter("dbg_s16b", [H, TOK], F32, isOutput=True)
        dbg_sc = nc.declare_dram_parameter("dbg_sc", [128, 8, TOK], F32, isOutput=True)
        dbg_ytn = nc.declare_dram_parameter("dbg_ytn", [128, 8, TOK], F16, isOutput=True)

    # ---- internal DRAM (separate per-b tensors: dep tracking is
    #      tensor-granular, shared tensors serialize across batches) ----
    g_scr = nc.dram_tensor("g_scr", [B, HP, TB], F32)
    # per-shard A2A payload: 128 raw y rows + 2 row-sum rows (bf16 for range)
    y_loc = [nc.dram_tensor(f"y_loc{b}", [NC, ROWS + HP, TOK], BF16) for b in range(B)]
    y_tok = [nc.dram_tensor(f"y_tok{b}", [NC, ROWS + HP, TOK], BF16) for b in range(B)]
    rfac = [nc.dram_tensor(f"rfac{b}", [H, TOK], F32) for b in range(B)]
    warm_in = nc.dram_tensor("warm_in", [1, 64], F32)
    warm_out = nc.dram_tensor("warm_out", [NC, 64], F32, addr_space="Shared")

    groups = [list(range(NC))]

    with tile.TileContext(nc) as tc:
        with (
            tc.tile_pool(name="const", bufs=1) as constp,
            tc.tile_pool(name="xt", bufs=2) as xtp,
            tc.tile_pool(name="vt", bufs=1) as vtp,
            tc.tile_pool(name="rope", bufs=2) as ropep,
            tc.tile_pool(name="att", bufs=4) as attp,
            tc.tile_pool(name="gz", bufs=2) as gzp,
            tc.tile_pool(name="nb", bufs=4) as nbp,
            tc.tile_pool(name="yout", bufs=3) as youtp,
            tc.tile_pool(name="fin", bufs=2) as finp,
            tc.tile_pool(name="psum", bufs=2, space="PSUM") as psp,
            tc.tile_pool(name="psum_s", bufs=2, space="PSUM") as psps,
        ):
            # ---- warmup collective: absorbs cross-core start skew ----
            wt = constp.tile([1, 64], F32, tag="warm")
            nc.sync.dma_start(out=wt[:], in_=rope2[0:1, 0:64])
            nc.sync.dma_start(out=warm_in[:], in_=wt[:])
            nc.gpsimd.collective_compute(
                "AllGather", mybir.AluOpType.bypass, replica_groups=groups,
                ins=[warm_in[:]], outs=[warm_out[:]],
            )

            # ---- constants ----
            wq_sb = constp.tile([128, 8, ROWS], F16, tag="wq")
            wk_sb = constp.tile([128, 8, ROWS], F16, tag="wk")
            wv_sb = constp.tile([128, 8, ROWS], F16, tag="wv")
            gw_sb = constp.tile([128, 8, HP], F16, tag="gw")
            nc.sync.dma_start(out=wq_sb[:], in_=Wq.rearrange("(t p) m -> p t m", p=128))
            nc.sync.dma_start(out=wk_sb[:], in_=Wk.rearrange("(t p) m -> p t m", p=128))
            nc.sync.dma_start(out=wv_sb[:], in_=Wv.rearrange("(t p) m -> p t m", p=128))
            nc.sync.dma_start(out=gw_sb[:], in_=gw.rearrange("(t p) m -> p t m", p=128))
            gb_sb = constp.tile([HP, 1], F32, tag="gb")
            nc.sync.dma_start(out=gb_sb[:], in_=gb[:])
            # [Rdiag | Roff] side by side for paired near-band updates
            rdo = constp.tile([128, 256], F32, tag="rdo")
            nc.sync.dma_start(out=rdo[:, 0:128], in_=Rdiag[:])
            nc.sync.dma_start(out=rdo[:, 128:256], in_=Roff[:])
            wp_sb = constp.tile([128, 8, C], F16, tag="wp")
            nc.sync.dma_start(out=wp_sb[:], in_=Wproj.rearrange("(t p) m -> p t m", p=128))
            ident = constp.tile([128, 64], F16, tag="id")
            make_identity(nc, ident[0:64, :])
            make_identity(nc, ident[64:128, :])

            # cos/sin from host-prewrapped phases (ACT Sin valid on [-pi,pi]):
            # Sin(rope2)=cos(pos); Sin(rope2s)=sign-baked sin(pos)
            rp2 = gzp.tile([128, T], F32, tag="big8k")
            rp2s = gzp.tile([128, T], F32, tag="big8k")
            nc.sync.dma_start(out=rp2[:], in_=rope2[:])
            nc.sync.dma_start(out=rp2s[:], in_=rope2s[:])
            cos2 = constp.tile([128, T], F32, tag="cos2")
            sin2 = constp.tile([128, T], F32, tag="sin2")
            nc.scalar.activation(cos2[:], rp2[:], mybir.ActivationFunctionType.Sin)
            nc.scalar.activation(sin2[:], rp2s[:], mybir.ActivationFunctionType.Sin)

            # persistent per-kernel activations
            qT = constp.tile([128, B * T], F16, tag="qT")
            kT = constp.tile([128, B * T], F16, tag="kT")
            vaug = constp.tile([128, B * HP * NKT, D + 1], BF16, tag="vaug")
            nc.vector.memset(vaug[:, :, D:D + 1], 1.0)

            # ============ phase 1: projections for all batches ============
            for b in range(B):
                vT = vtp.tile([128, TB], F16, tag="vT")
                gzb = gzp.tile([HP, TB], F32, tag="big8k")
                for ch in range(NPC):
                    tok0 = b * TB + ch * PCH
                    xt = xtp.tile([128, 8, PCH], F16, tag="xt")
                    nc.sync.dma_start(
                        out=xt[:],
                        in_=xT.rearrange("(t p) n -> p t n", p=128)[:, :, tok0:tok0 + PCH],
                    )
                    rcs = slice(ch * PCH, (ch + 1) * PCH)   # within-batch cols
                    acs = slice(tok0, tok0 + PCH)           # absolute cols

                    for which, w_sb in (("q", wq_sb), ("k", wk_sb)):
                        ps = psp.tile([128, PCH], F32, tag="mm")
                        for kt in range(8):
                            nc.tensor.matmul(ps[:], w_sb[:, kt, :], xt[:, kt, :],
                                             start=(kt == 0), stop=(kt == 7))
                        # rope: dst = ps*cos2 + swap(ps)*sin2
                        qf = ropep.tile([128, PCH], F32, tag="qf")
                        nc.vector.tensor_copy(qf[:], ps[:])
                        qsw = ropep.tile([128, PCH], F32, tag="qsw")
                        for blk in range(4):
                            srow = (blk ^ 1) * 32
                            nc.sync.dma_start(out=qsw[blk * 32:(blk + 1) * 32, :],
                                              in_=qf[srow:srow + 32, :])
                        nc.vector.tensor_mul(qf[:], qf[:], cos2[:, rcs])
                        nc.vector.tensor_mul(qsw[:], qsw[:], sin2[:, rcs])
                        dst = qT if which == "q" else kT
                        nc.vector.tensor_add(dst[:, acs], qf[:], qsw[:])

                    ps = psp.tile([128, PCH], F32, tag="mm")
                    for kt in range(8):
                        nc.tensor.matmul(ps[:], wv_sb[:, kt, :], xt[:, kt, :],
                                         start=(kt == 0), stop=(kt == 7))
                    nc.vector.tensor_copy(vT[:, rcs], ps[:])

                    gps = psp.tile([HP, PCH], F32, tag="mm")
                    for kt in range(8):
                        nc.tensor.matmul(gps[:], gw_sb[:, kt, :], xt[:, kt, :],
                                         start=(kt == 0), stop=(kt == 7))
                    nc.vector.tensor_copy(gzb[:, rcs], gps[:])

                # V to token-major with ones column (PE transpose)
                for h in range(HP):
                    for tt in range(NKT):
                        tp = psps.tile([128, D], F16, tag="s")
                        nc.tensor.transpose(
                            tp[:], vT[h * D:(h + 1) * D, tt * KT:(tt + 1) * KT],
                            ident[h * D:(h + 1) * D, :])
                        nc.vector.tensor_copy(vaug[:, (b * HP + h) * NKT + tt, 0:D], tp[:])

                # softplus(z) = ln(1 + exp(z)) batched per batch
                gex = gzp.tile([HP, TB], F32, tag="big8k")
                nc.scalar.activation(gex[:], gzb[:], mybir.ActivationFunctionType.Exp,
                                     bias=gb_sb[:], scale=1.0)
                nc.scalar.activation(gex[:], gex[:], mybir.ActivationFunctionType.Ln,
                                     bias=1.0, scale=1.0)
                nc.sync.dma_start(out=g_scr[b], in_=gex[:])

            # ============ phase 2: attention per batch + AllToAll ============
            for b in range(B):
                for qc in range(NQC):
                    yps = [psp.tile([D + 1, QCH], F32, tag="y", name=f"yps{_h}") for _h in range(HP)]
                    # band strips for this q-chunk, computed off the critical
                    # path: slot m (j0 = 4qc-1+m) holds [diag(j0) | off(j0)]
                    # = [g(j0*128+i)*Rdiag | g((j0+1)*128+i)*Roff], f32.
                    strips = []
                    for h in range(HP):
                        gbc = nbp.tile([128, QCH], F32, tag="gbc", name=f"gbc{h}")
                        nc.sync.dma_start(
                            out=gbc[:],
                            in_=g_scr[b, h, qc * QCH:(qc + 1) * QCH]
                            .unsqueeze(0).to_broadcast((128, QCH)))
                        stc = nbp.tile([128, 5, 256], F32, tag="stc", name=f"stc{h}")
                        gv = gbc[:].rearrange("p (m n) -> p m n", n=KT)
                        nc.vector.tensor_mul(
                            stc[:, 1:5, 0:KT], gv,
                            rdo[:, None, 0:KT].to_broadcast((128, 4, KT)))
                        nc.vector.tensor_mul(
                            stc[:, 0:4, KT:256], gv,
                            rdo[:, None, KT:256].to_broadcast((128, 4, KT)))
                        strips.append(stc)
                    njt = 4 * qc + 4
                    pend = [None, None]
                    for j0 in range(njt):
                        r = j0 - 4 * qc
                        width = QCH if r < 0 else (4 - r) * KT
                        loc0 = max(r, 0) * KT
                        spair = psps.tile([128, HP, QCH], F32, tag="s")
                        for h in range(HP):
                            hr = slice(h * D, (h + 1) * D)
                            nc.tensor.matmul(
                                spair[:, h, 0:width],
                                kT[hr, b * TB + j0 * KT:b * TB + (j0 + 1) * KT],
                                qT[hr, b * TB + qc * QCH + loc0:b * TB + (qc + 1) * QCH],
                                start=True, stop=True,
                                tile_position=(h * D, 0))
                        # near-band additive gravity term (+ causal mask)
                        has_d = r >= 0
                        has_o = -1 <= r <= 2
                        if has_d or has_o:
                            m = r + 1
                            c0 = 0 if has_d else KT
                            c1 = 256 if has_o else KT
                            lc = 0 if has_d else (KT if r >= 0 else 0)
                            for h in range(HP):
                                nc.vector.tensor_add(
                                    spair[:, h, lc:lc + (c1 - c0)],
                                    spair[:, h, lc:lc + (c1 - c0)],
                                    strips[h][:, m, c0:c1])
                        et = attp.tile([128, HP, QCH], BF16, tag="et")
                        nc.scalar.activation(et[:, :, 0:width], spair[:, :, 0:width],
                                             mybir.ActivationFunctionType.Exp)
                        for h in range(HP):
                            if pend[h] is not None:
                                nc.tensor.matmul(*pend[h][0], start=pend[h][1], stop=False)
                            pend[h] = ((yps[h][:, loc0:QCH],
                                        vaug[:, (b * HP + h) * NKT + j0, :],
                                        et[:, h, 0:width]), j0 == 0)
                    for h in range(HP):
                        nc.tensor.matmul(*pend[h][0], start=pend[h][1], stop=True)
                        # stage raw y + sums into the A2A payload (normalize
                        # happens on the receiving side, off the critical path)
                        yst = youtp.tile([D + 1, QCH], BF16, tag="yst")
                        nc.vector.tensor_copy(yst[:], yps[h][:])
                        for sh in range(2):
                            nc.sync.dma_start(
                                out=y_loc[b][2 * qc + sh, h * D:(h + 1) * D, :],
                                in_=yst[0:D, sh * TOK:(sh + 1) * TOK])
                            nc.sync.dma_start(
                                out=y_loc[b][2 * qc + sh, ROWS + h, :],
                                in_=yst[D:D + 1, sh * TOK:(sh + 1) * TOK])

                if DEBUG and b == 0:
                    nc.sync.dma_start(out=dbg_q[:], in_=qT[:, 0:TB])
                    nc.sync.dma_start(out=dbg_k[:], in_=kT[:, 0:TB])
                    nc.sync.dma_start(out=dbg_g[:], in_=g_scr[0])

                nc.gpsimd.collective_compute(
                    "AllToAll", mybir.AluOpType.bypass, replica_groups=groups,
                    ins=[y_loc[b][:]], outs=[y_tok[b][:]],
                )

            # ======== phase 3: output projection (my 256 tokens / batch) =====
            for b in range(B):
                yt = finp.tile([128, 8, TOK], BF16, tag="yt")
                nc.sync.dma_start(
                    out=yt[:], in_=y_tok[b][:, 0:ROWS, :].rearrange("s p n -> p s n"))
                sums16 = finp.tile([H, TOK], BF16, tag="sums16")
                nc.sync.dma_start(out=sums16[:], in_=y_tok[b][:, ROWS:ROWS + HP, :])
                rec16 = finp.tile([H, TOK], F32, tag="rec16")
                nc.vector.reciprocal(rec16[:], sums16[:])
                if DEBUG and b == 0:
                    nc.sync.dma_start(out=dbg_s16b[:], in_=rec16[:])
                nc.sync.dma_start(out=rfac[b][:], in_=rec16[:])
                scale = finp.tile([128, 8, TOK], F32, tag="scale")
                for h in range(HP):
                    nc.sync.dma_start(
                        out=scale[h * D:(h + 1) * D, :, :],
                        in_=rfac[b].rearrange("(s h) n -> s h n", h=HP)[:, h, :]
                        .unsqueeze(0).to_broadcast((D, 8, TOK)))
                ytn = finp.tile([128, 8, TOK], F16, tag="ytn")
                nc.vector.tensor_mul(ytn[:], yt[:], scale[:])
                if DEBUG and b == 0:
                    nc.sync.dma_start(out=dbg_yt[:], in_=y_tok[0][:])
                    nc.sync.dma_start(out=dbg_sc[:], in_=scale[:])
                    nc.sync.dma_start(out=dbg_ytn[:], in_=ytn[:])
                for co in range(8):
                    fps = psp.tile([128, TOK], F32, tag="mm")
                    for kt in range(8):
                        nc.tensor.matmul(fps[:], wp_sb[:, kt, co * 128:(co + 1) * 128],
                                         ytn[:, kt, :], start=(kt == 0), stop=(kt == 7))
                    ot = finp.tile([128, TOK], F32, tag="ot")
                    nc.vector.tensor_copy(ot[:], fps[:])
                    nc.sync.dma_start(out=out[b, co], in_=ot[:])

    nc.compile()
    nc.finalize()
    return nc


def _host_prep(x, rope, W_attn, W_proj, g_w, g_b):
    xT = np.ascontiguousarray(x.reshape(B * T, C).T).astype(np.float16)
    ropeT = np.ascontiguousarray(rope.T.astype(np.float64))  # [64, T]
    rope2 = np.concatenate([ropeT, ropeT], axis=0)  # [128, T]
    rope2s = rope2.copy()
    rr = np.arange(128)
    rope2s[(rr % 64) < 32] *= -1.0

    def wrap(a):  # into [-pi, pi] for the ACT Sin LUT
        return ((a + np.pi) % (2 * np.pi) - np.pi).astype(np.float32)

    rope2 = wrap(rope2 + np.pi / 2)   # Sin(rope2) == cos(pos)
    rope2s = wrap(rope2s)             # Sin(rope2s) == sign-baked sin(pos)
    Wproj16 = W_proj.astype(np.float16)

    idx = np.arange(128)
    ii, jj = idx[None, :], idx[:, None]  # Rdiag[j, i]
    rdiag = np.where(ii >= jj, 0.1 * np.maximum(64.0 - (ii - jj), 0.0), -1e9).astype(np.float32)
    roff = (0.1 * np.maximum(jj - ii - 64.0, 0.0)).astype(np.float32)

    in_maps = []
    for c in range(NC):
        h0 = HP * c
        cols = slice(h0 * D, (h0 + HP) * D)
        in_maps.append({
            "xT": xT,
            "rope2": rope2,
            "rope2s": rope2s,
            "Wq": np.ascontiguousarray(W_attn[:, cols] * 0.125).astype(np.float16),
            "Wk": np.ascontiguousarray(W_attn[:, C:][:, cols]).astype(np.float16),
            "Wv": np.ascontiguousarray(W_attn[:, 2 * C:][:, cols]).astype(np.float16),
            "gw": np.ascontiguousarray(g_w[:, h0:h0 + HP]).astype(np.float16),
            "gb": np.ascontiguousarray(g_b[h0:h0 + HP].reshape(HP, 1)).astype(np.float32),
            "Rdiag": rdiag,
            "Roff": roff,
            "Wproj": Wproj16,
        })
    return in_maps


def kernel(x, rope, W_attn, W_proj, g_w, g_b):
    global LAST_RESULT
    x = np.asarray(x, dtype=np.float32)
    rope = np.asarray(rope, dtype=np.float32)
    W_attn = np.asarray(W_attn, dtype=np.float32)
    W_proj = np.asarray(W_proj, dtype=np.float32)
    g_w = np.asarray(g_w, dtype=np.float32)
    g_b = np.asarray(g_b, dtype=np.float32)

    if "nc" not in _CACHED:
        _CACHED["nc"] = build_nc()
    nc = _CACHED["nc"]
    in_maps = _host_prep(x, rope, W_attn, W_proj, g_w, g_b)
    res = run_bass_kernel_spmd(nc, in_maps, list(range(NC)),
                               trace=bool(os.environ.get("AGA_TRACE")))
    LAST_RESULT = res
    # assemble: core c holds out[b, co, p, t] for tokens c*256..(c+1)*256
    full = np.empty((B, T, C), dtype=np.float32)
    for c in range(NC):
        oc = res.results[c]["out"]  # [B, 8, 128, TOK]
        for b in range(B):
            full[b, c * TOK:(c + 1) * TOK, :] = oc[b].reshape(C, TOK).T
    return full


# revision 34
# speedup vs baseline: 1.0951x; 1.0951x over previous
"""AdaptiveGravityAttention on 8 TRN2 NeuronCores (Bass/Tile, SPMD).

Sharding: heads across cores (2 heads/core, all batches), token-parallel
final projection via per-batch AllToAll. All matmuls fp16/bf16 inputs with
fp32 PSUM accumulation.

Math notes:
- softmax rows are rebased by exp(+6.4*g[i]) so the gravity bias becomes an
  additive band term 0.1*g[i]*max(64-dist,0) that is zero outside |i-j|<64;
  causal masking rides the same term via a -1e9 entry (g>0 always).
- logits are bounded (~|6|+6.4*g<=40) so exp needs no running max; exp
  outputs stored bf16 (range), q/k/v/x/W stored fp16 (precision).

Structure (v2):
- phase 1: projections + rope + V-transpose for ALL batches (keeps PE dense)
- phase 2: per batch: attention (head-paired S matmuls on disjoint PE row
  groups), batched reciprocal, normalize, AllToAll
- phase 3: output projection for this core's token slice of every batch
- SBUF-only elementwise (rope sin-term, near-band product, normalize mul)
  runs on the otherwise-idle GpSimd engine; DVE keeps all PSUM traffic.
"""
import os
import sys

sys.path.insert(0, "/opt/trn_rl_repo")

import numpy as np

import concourse.bass as bass  # noqa: F401
import concourse.mybir as mybir
import concourse.tile as tile
from concourse import bacc
from concourse.masks import make_identity
from concourse.bass_utils import run_bass_kernel_spmd

B, T, C = 4, 2048, 1024
H, D = 16, 64
NC = 8
HP = H // NC          # heads per core = 2
ROWS = HP * D         # 128 q/k/v rows per core
TB = T                # tokens per batch
QCH = 512             # q-chunk width
NQC = TB // QCH       # 4
KT = 128              # k-tile
NKT = TB // KT        # 16
PCH = 512             # projection token chunk
NPC = TB // PCH       # 4
TOK = T // NC         # tokens per core per batch for out-proj = 256

F16 = mybir.dt.float16
BF16 = mybir.dt.bfloat16
F32 = mybir.dt.float32

_CACHED = {}
LAST_RESULT = None
DEBUG = False


def build_nc():
    nc = bacc.Bacc("TRN2", target_bir_lowering=False, num_devices=NC)

    # ---- dram parameters (per-core shards prepared on host) ----
    xT = nc.declare_dram_parameter("xT", [C, B * T], F16, isOutput=False)
    rope2 = nc.declare_dram_parameter("rope2", [128, T], F32, isOutput=False)
    rope2s = nc.declare_dram_parameter("rope2s", [128, T], F32, isOutput=False)
    Wq = nc.declare_dram_parameter("Wq", [C, ROWS], F16, isOutput=False)
    Wk = nc.declare_dram_parameter("Wk", [C, ROWS], F16, isOutput=False)
    Wv = nc.declare_dram_parameter("Wv", [C, ROWS], F16, isOutput=False)
    gw = nc.declare_dram_parameter("gw", [C, HP], F16, isOutput=False)
    gb = nc.declare_dram_parameter("gb", [HP, 1], F32, isOutput=False)
    Rdiag = nc.declare_dram_parameter("Rdiag", [128, 128], F32, isOutput=False)
    Roff = nc.declare_dram_parameter("Roff", [128, 128], F32, isOutput=False)
    Wproj = nc.declare_dram_parameter("Wproj", [C, C], F16, isOutput=False)
    out = nc.declare_dram_parameter("out", [B, 8, 128, TOK], F32, isOutput=True)
    if DEBUG:
        dbg_q = nc.declare_dram_parameter("dbg_q", [128, TB], F16, isOutput=True)
        dbg_k = nc.declare_dram_parameter("dbg_k", [128, TB], F16, isOutput=True)
        dbg_g = nc.declare_dram_parameter("dbg_g", [HP, TB], F32, isOutput=True)
        dbg_yt = nc.declare_dram_parameter("dbg_yt", [NC, ROWS + HP, TOK], BF16, isOutput=True)
        dbg_s16 = nc.declare_dram_parameter("dbg_s16", [H, TOK], F32, isOutput=True)
        dbg_s16b = nc.declare_dram_parameter("dbg_s16b", [H, TOK], F32, isOutput=True)
        dbg_sc = nc.declare_dram_parameter("dbg_sc", [128, 8, TOK], F32, isOutput=True)
        dbg_ytn = nc.declare_dram_parameter("dbg_ytn", [128, 8, TOK], F16, isOutput=True)

    # ---- internal DRAM (separate per-b tensors: dep tracking is
    #      tensor-granular, shared tensors serialize across batches) ----
    g_scr = nc.dram_tensor("g_scr", [B, HP, TB], F32)
    # per-shard A2A payload: 128 raw y rows + 2 row-sum rows (bf16 for range)
    y_loc = [nc.dram_tensor(f"y_loc{b}", [NC, ROWS + HP, TOK], BF16) for b in range(B)]
    y_tok = [nc.dram_tensor(f"y_tok{b}", [NC, ROWS + HP, TOK], BF16) for b in range(B)]
    rfac = [nc.dram_tensor(f"rfac{b}", [H, TOK], F32) for b in range(B)]
    warm_in = nc.dram_tensor("warm_in", [1, 64], F32)
    warm_out = nc.dram_tensor("warm_out", [NC, 64], F32, addr_space="Shared")

    groups = [list(range(NC))]

    with tile.TileContext(nc) as tc:
        with (
            tc.tile_pool(name="const", bufs=1) as constp,
            tc.tile_pool(name="xt", bufs=3) as xtp,
            tc.tile_pool(name="sc1", bufs=1) as scp,
            tc.tile_pool(name="gbcp", bufs=2) as gbcp,
            tc.tile_pool(name="vt", bufs=1) as vtp,
            tc.tile_pool(name="rope", bufs=2) as ropep,
            tc.tile_pool(name="att", bufs=4) as attp,
            tc.tile_pool(name="gz", bufs=2) as gzp,
            tc.tile_pool(name="nb", bufs=4) as nbp,
            tc.tile_pool(name="yout", bufs=3) as youtp,
            tc.tile_pool(name="fin", bufs=2) as finp,
            tc.tile_pool(name="psum", bufs=2, space="PSUM") as psp,
            tc.tile_pool(name="psum_s", bufs=2, space="PSUM") as psps,
        ):
            # ---- warmup collective: absorbs cross-core start skew ----
            wt = constp.tile([1, 64], F32, tag="warm")
            nc.sync.dma_start(out=wt[:], in_=rope2[0:1, 0:64])
            nc.sync.dma_start(out=warm_in[:], in_=wt[:])
            nc.gpsimd.collective_compute(
                "AllGather", mybir.AluOpType.bypass, replica_groups=groups,
                ins=[warm_in[:]], outs=[warm_out[:]],
            )

            # ---- constants ----
            wq_sb = constp.tile([128, 8, ROWS], F16, tag="wq")
            wk_sb = constp.tile([128, 8, ROWS], F16, tag="wk")
            wv_sb = constp.tile([128, 8, ROWS], F16, tag="wv")
            gw_sb = constp.tile([128, 8, HP], F16, tag="gw")
            nc.sync.dma_start(out=wq_sb[:], in_=Wq.rearrange("(t p) m -> p t m", p=128))
            nc.sync.dma_start(out=wk_sb[:], in_=Wk.rearrange("(t p) m -> p t m", p=128))
            nc.sync.dma_start(out=wv_sb[:], in_=Wv.rearrange("(t p) m -> p t m", p=128))
            nc.sync.dma_start(out=gw_sb[:], in_=gw.rearrange("(t p) m -> p t m", p=128))
            gb_sb = constp.tile([HP, 1], F32, tag="gb")
            nc.sync.dma_start(out=gb_sb[:], in_=gb[:])
            # [Rdiag | Roff] side by side for paired near-band updates
            rdo = constp.tile([128, 256], F32, tag="rdo")
            nc.sync.dma_start(out=rdo[:, 0:128], in_=Rdiag[:])
            nc.sync.dma_start(out=rdo[:, 128:256], in_=Roff[:])
            wp_sb = constp.tile([128, 8, C], F16, tag="wp")
            nc.sync.dma_start(out=wp_sb[:], in_=Wproj.rearrange("(t p) m -> p t m", p=128))
            ident = constp.tile([128, 64], F16, tag="id")
            make_identity(nc, ident[0:64, :])
            make_identity(nc, ident[64:128, :])

            # cos/sin from host-prewrapped phases (ACT Sin valid on [-pi,pi]):
            # Sin(rope2)=cos(pos); Sin(rope2s)=sign-baked sin(pos)
            rp2 = gzp.tile([128, T], F32, tag="big8k")
            rp2s = gzp.tile([128, T], F32, tag="big8k")
            nc.sync.dma_start(out=rp2[:], in_=rope2[:])
            nc.sync.dma_start(out=rp2s[:], in_=rope2s[:])
            cos2 = constp.tile([128, T], F32, tag="cos2")
            sin2 = constp.tile([128, T], F32, tag="sin2")
            nc.scalar.activation(cos2[:], rp2[:], mybir.ActivationFunctionType.Sin)
            nc.scalar.activation(sin2[:], rp2s[:], mybir.ActivationFunctionType.Sin)

            # persistent per-kernel activations
            qT = constp.tile([128, B * T], F16, tag="qT")
            kT = constp.tile([128, B * T], F16, tag="kT")
            vaug = constp.tile([128, B * HP * NKT, D + 1], BF16, tag="vaug")
            nc.vector.memset(vaug[:, :, D:D + 1], 1.0)

            # ============ phase 1: projections for all batches ============
            for b in range(B):
                vT = vtp.tile([128, TB], F16, tag="vT")
                gzb = gzp.tile([HP, TB], F32, tag="big8k")
                for ch in range(NPC):
                    tok0 = b * TB + ch * PCH
                    xt = xtp.tile([128, 8, PCH], F16, tag="xt")
                    nc.sync.dma_start(
                        out=xt[:],
                        in_=xT.rearrange("(t p) n -> p t n", p=128)[:, :, tok0:tok0 + PCH],
                    )
                    rcs = slice(ch * PCH, (ch + 1) * PCH)   # within-batch cols
                    acs = slice(tok0, tok0 + PCH)           # absolute cols

                    for which, w_sb in (("q", wq_sb), ("k", wk_sb)):
                        ps = psp.tile([128, PCH], F32, tag="mm")
                        for kt in range(8):
                            nc.tensor.matmul(ps[:], w_sb[:, kt, :], xt[:, kt, :],
                                             start=(kt == 0), stop=(kt == 7))
                        # rope: dst = ps*cos2 + swap(ps)*sin2
                        qf = ropep.tile([128, PCH], F32, tag="qf")
                        nc.vector.tensor_copy(qf[:], ps[:])
                        qsw = ropep.tile([128, PCH], F32, tag="qsw")
                        for blk in range(4):
                            srow = (blk ^ 1) * 32
                            nc.sync.dma_start(out=qsw[blk * 32:(blk + 1) * 32, :],
                                              in_=qf[srow:srow + 32, :])
                        nc.vector.tensor_mul(qf[:], qf[:], cos2[:, rcs])
                        nc.vector.tensor_mul(qsw[:], qsw[:], sin2[:, rcs])
                        dst = qT if which == "q" else kT
                        nc.vector.tensor_add(dst[:, acs], qf[:], qsw[:])

                    ps = psp.tile([128, PCH], F32, tag="mm")
                    for kt in range(8):
                        nc.tensor.matmul(ps[:], wv_sb[:, kt, :], xt[:, kt, :],
                                         start=(kt == 0), stop=(kt == 7))
                    nc.vector.tensor_copy(vT[:, rcs], ps[:])

                    gps = psp.tile([HP, PCH], F32, tag="mm")
                    for kt in range(8):
                        nc.tensor.matmul(gps[:], gw_sb[:, kt, :], xt[:, kt, :],
                                         start=(kt == 0), stop=(kt == 7))
                    nc.vector.tensor_copy(gzb[:, rcs], gps[:])

                # V to token-major with ones column (PE transpose)
                for h in range(HP):
                    for tt in range(NKT):
                        tp = psps.tile([128, D], F16, tag="s")
                        nc.tensor.transpose(
                            tp[:], vT[h * D:(h + 1) * D, tt * KT:(tt + 1) * KT],
                            ident[h * D:(h + 1) * D, :])
                        nc.vector.tensor_copy(vaug[:, (b * HP + h) * NKT + tt, 0:D], tp[:])

                # softplus(z) = ln(1 + exp(z)) batched per batch
                gex = gzp.tile([HP, TB], F32, tag="big8k")
                nc.scalar.activation(gex[:], gzb[:], mybir.ActivationFunctionType.Exp,
                                     bias=gb_sb[:], scale=1.0)
                nc.scalar.activation(gex[:], gex[:], mybir.ActivationFunctionType.Ln,
                                     bias=1.0, scale=1.0)
                nc.sync.dma_start(out=g_scr[b], in_=gex[:])

            # ============ phase 2: attention per batch + AllToAll ============
            for b in range(B):
                for qc in range(NQC):
                    yps = [psp.tile([D + 1, QCH], F32, tag="y", name=f"yps{_h}") for _h in range(HP)]
                    # band strips for this q-chunk, computed off the critical
                    # path: slot m (j0 = 4qc-1+m) holds [diag(j0) | off(j0)]
                    # = [g(j0*128+i)*Rdiag | g((j0+1)*128+i)*Roff], f32.
                    strips = []
                    for h in range(HP):
                        gbc = gbcp.tile([128, QCH], F32, tag="gbc", name=f"gbc{h}")
                        nc.sync.dma_start(
                            out=gbc[:],
                            in_=g_scr[b, h, qc * QCH:(qc + 1) * QCH]
                            .unsqueeze(0).to_broadcast((128, QCH)))
                        stc = nbp.tile([128, 5, 256], F32, tag="stc", name=f"stc{h}")
                        gv = gbc[:].rearrange("p (m n) -> p m n", n=KT)
                        nc.vector.tensor_mul(
                            stc[:, 1:5, 0:KT], gv,
                            rdo[:, None, 0:KT].to_broadcast((128, 4, KT)))
                        nc.vector.tensor_mul(
                            stc[:, 0:4, KT:256], gv,
                            rdo[:, None, KT:256].to_broadcast((128, 4, KT)))
                        strips.append(stc)
                    njt = 4 * qc + 4
                    pend = [None, None]
                    for j0 in range(njt):
                        r = j0 - 4 * qc
                        width = QCH if r < 0 else (4 - r) * KT
                        loc0 = max(r, 0) * KT
                        spair = psps.tile([128, HP, QCH], F32, tag="s")
                        for h in range(HP):
                            hr = slice(h * D, (h + 1) * D)
                            nc.tensor.matmul(
                                spair[:, h, 0:width],
                                kT[hr, b * TB + j0 * KT:b * TB + (j0 + 1) * KT],
                                qT[hr, b * TB + qc * QCH + loc0:b * TB + (qc + 1) * QCH],
                                start=True, stop=True,
                                tile_position=(h * D, 0))
                        # near-band additive gravity term (+ causal mask)
                        has_d = r >= 0
                        has_o = -1 <= r <= 2
                        if has_d or has_o:
                            m = r + 1
                            c0 = 0 if has_d else KT
                            c1 = 256 if has_o else KT
                            lc = 0 if has_d else (KT if r >= 0 else 0)
                            for h in range(HP):
                                nc.vector.tensor_add(
                                    spair[:, h, lc:lc + (c1 - c0)],
                                    spair[:, h, lc:lc + (c1 - c0)],
                                    strips[h][:, m, c0:c1])
                        et = attp.tile([128, HP, QCH], BF16, tag="et")
                        nc.scalar.activation(et[:, :, 0:width], spair[:, :, 0:width],
                                             mybir.ActivationFunctionType.Exp)
                        for h in range(HP):
                            if pend[h] is not None:
                                nc.tensor.matmul(*pend[h][0], start=pend[h][1], stop=False)
                            pend[h] = ((yps[h][:, loc0:QCH],
                                        vaug[:, (b * HP + h) * NKT + j0, :],
                                        et[:, h, 0:width]), j0 == 0)
                    for h in range(HP):
                        nc.tensor.matmul(*pend[h][0], start=pend[h][1], stop=True)
                        # stage raw y + sums into the A2A payload (normalize
                        # happens on the receiving side, off the critical path)
                        yst = youtp.tile([D + 1, QCH], BF16, tag="yst")
                        nc.vector.tensor_copy(yst[:], yps[h][:])
                        for sh in range(2):
                            nc.sync.dma_start(
                                out=y_loc[b][2 * qc + sh, h * D:(h + 1) * D, :],
                                in_=yst[0:D, sh * TOK:(sh + 1) * TOK])
                            nc.sync.dma_start(
                                out=y_loc[b][2 * qc + sh, ROWS + h, :],
                                in_=yst[D:D + 1, sh * TOK:(sh + 1) * TOK])

                if DEBUG and b == 0:
                    nc.sync.dma_start(out=dbg_q[:], in_=qT[:, 0:TB])
                    nc.sync.dma_start(out=dbg_k[:], in_=kT[:, 0:TB])
                    nc.sync.dma_start(out=dbg_g[:], in_=g_scr[0])

                nc.gpsimd.collective_compute(
                    "AllToAll", mybir.AluOpType.bypass, replica_groups=groups,
                    ins=[y_loc[b][:]], outs=[y_tok[b][:]],
                )

            # ======== phase 3: output projection (my 256 tokens / batch) =====
            for b in range(B):
                yt = finp.tile([128, 8, TOK], BF16, tag="yt")
                nc.sync.dma_start(
                    out=yt[:], in_=y_tok[b][:, 0:ROWS, :].rearrange("s p n -> p s n"))
                sums16 = finp.tile([H, TOK], BF16, tag="sums16")
                nc.sync.dma_start(out=sums16[:], in_=y_tok[b][:, ROWS:ROWS + HP, :])
                rec16 = finp.tile([H, TOK], F32, tag="rec16")
                nc.vector.reciprocal(rec16[:], sums16[:])
                if DEBUG and b == 0:
                    nc.sync.dma_start(out=dbg_s16b[:], in_=rec16[:])
                nc.sync.dma_start(out=rfac[b][:], in_=rec16[:])
                scale = scp.tile([128, 8, TOK], F32, tag="scale")
                for h in range(HP):
                    nc.sync.dma_start(
                        out=scale[h * D:(h + 1) * D, :, :],
                        in_=rfac[b].rearrange("(s h) n -> s h n", h=HP)[:, h, :]
                        .unsqueeze(0).to_broadcast((D, 8, TOK)))
                ytn = finp.tile([128, 8, TOK], F16, tag="ytn")
                nc.vector.tensor_mul(ytn[:], yt[:], scale[:])
                if DEBUG and b == 0:
                    nc.sync.dma_start(out=dbg_yt[:], in_=y_tok[0][:])
                    nc.sync.dma_start(out=dbg_sc[:], in_=scale[:])
                    nc.sync.dma_start(out=dbg_ytn[:], in_=ytn[:])
                for co in range(8):
                    fps = psp.tile([128, TOK], F32, tag="mm")
                    for kt in range(8):
                        nc.tensor.matmul(fps[:], wp_sb[:, kt, co * 128:(co + 1) * 128],
                                         ytn[:, kt, :], start=(kt == 0), stop=(kt == 7))
                    ot = finp.tile([128, TOK], F32, tag="ot")
                    nc.vector.tensor_copy(ot[:], fps[:])
                    nc.sync.dma_start(out=out[b, co], in_=ot[:])

    nc.compile()
    nc.finalize()
    return nc


def _host_prep(x, rope, W_attn, W_proj, g_w, g_b):
    xT = np.ascontiguousarray(x.reshape(B * T, C).T).astype(np.float16)
    ropeT = np.ascontiguousarray(rope.T.astype(np.float64))  # [64, T]
    rope2 = np.concatenate([ropeT, ropeT], axis=0)  # [128, T]
    rope2s = rope2.copy()
    rr = np.arange(128)
    rope2s[(rr % 64) < 32] *= -1.0

    def wrap(a):  # into [-pi, pi] for the ACT Sin LUT
        return ((a + np.pi) % (2 * np.pi) - np.pi).astype(np.float32)

    rope2 = wrap(rope2 + np.pi / 2)   # Sin(rope2) == cos(pos)
    rope2s = wrap(rope2s)             # Sin(rope2s) == sign-baked sin(pos)
    Wproj16 = W_proj.astype(np.float16)

    idx = np.arange(128)
    ii, jj = idx[None, :], idx[:, None]  # Rdiag[j, i]
    rdiag = np.where(ii >= jj, 0.1 * np.maximum(64.0 - (ii - jj), 0.0), -1e9).astype(np.float32)
    roff = (0.1 * np.maximum(jj - ii - 64.0, 0.0)).astype(np.float32)

    in_maps = []
    for c in range(NC):
        h0 = HP * c
        cols = slice(h0 * D, (h0 + HP) * D)
        in_maps.append({
            "xT": xT,
            "rope2": rope2,
            "rope2s": rope2s,
            "Wq": np.ascontiguousarray(W_attn[:, cols] * 0.125).astype(np.float16),
            "Wk": np.ascontiguousarray(W_attn[:, C:][:, cols]).astype(np.float16),
            "Wv": np.ascontiguousarray(W_attn[:, 2 * C:][:, cols]).astype(np.float16),
            "gw": np.ascontiguousarray(g_w[:, h0:h0 + HP]).astype(np.float16),
            "gb": np.ascontiguousarray(g_b[h0:h0 + HP].reshape(HP, 1)).astype(np.float32),
            "Rdiag": rdiag,
            "Roff": roff,
            "Wproj": Wproj16,
        })
    return in_maps


def kernel(x, rope, W_attn, W_proj, g_w, g_b):
    global LAST_RESULT
    x = np.asarray(x, dtype=np.float32)
    rope = np.asarray(rope, dtype=np.float32)
    W_attn = np.asarray(W_attn, dtype=np.float32)
    W_proj = np.asarray(W_proj, dtype=np.float32)
    g_w = np.asarray(g_w, dtype=np.float32)
    g_b = np.asarray(g_b, dtype=np.float32)

    if "nc" not in _CACHED:
        _CACHED["nc"] = build_nc()
    nc = _CACHED["nc"]
    in_maps = _host_prep(x, rope, W_attn, W_proj, g_w, g_b)
    res = run_bass_kernel_spmd(nc, in_maps, list(range(NC)),
                               trace=bool(os.environ.get("AGA_TRACE")))
    LAST_RESULT = res
    # assemble: core c holds out[b, co, p, t] for tokens c*256..(c+1)*256
    full = np.empty((B, T, C), dtype=np.float32)
    for c in range(NC):
        oc = res.results[c]["out"]  # [B, 8, 128, TOK]
        for b in range(B):
            full[b, c * TOK:(c + 1) * TOK, :] = oc[b].reshape(C, TOK).T
    return full
